# revision 13
# baseline (speedup 1.0000x reference)
"""Causal self-attention (B=4, T=2048, C=1024, H=16) on 8 NeuronCores.

Sharding: core c handles batch b = c//2 and head-half half = c%2 (8 heads,
512 channels). QKV projections are column-parallel, output projection is
row-parallel (Megatron); the two per-batch output partials are summed on host.

v2 design notes (vs the f32r baseline):
  - Activations (qT/kT/vAug/P/yT) are stored bf16: same PE stream rate as
    f32r (1 cyc/row) but no 4x penalty on <256-col moving operands, half the
    SBUF footprint, and 2x/4x DVE modes on elementwise ops.
  - yT lives in SBUF (no DRAM scratch round-trip, no phase-3 reload DMAs).
  - Biases are folded into the PSUM->SBUF drain as tensor_tensor adds with
    free-dim-broadcast operands (no bias matmuls on the PE).
  - Phase 2 uses 512-wide query chunks; scores for up to 3 key blocks land in
    one PSUM tile [128,1536] so one exp activation covers 3 blocks (ACT
    per-instruction overhead amortized). Softmax uses a fixed max of 0
    (scores ~N(0,1)) and the denominator comes out of the y matmul via an
    appended ones-column on v (vAug).
  - y-matmul emission lags score groups by 2 (software pipeline) so the PE
    never waits on the ACT engine's exp: PE stalls reset its DVFS ramp and
    halve its clock for 3us, so PE density is the top priority.
"""

import sys
import types

import numpy as np
from contextlib import ExitStack

import concourse.bass as bass
import concourse.mybir as mybir
import concourse.tile as tile
from concourse import bacc
from concourse.bass_utils import run_bass_kernel_spmd

# If the environment sets BASS_TRACE but ships only the antenv stub (no
# axon_hooks), run_bass_kernel_spmd would crash on import. Provide the
# graceful "no hook registered" fallback only when the real module is absent.
try:  # pragma: no cover
    import antenv.axon_hooks  # noqa: F401
except ImportError:  # pragma: no cover
    import antenv

    _stub = types.ModuleType("antenv.axon_hooks")
    _stub.get_axon_ntff_profile_hook = lambda: None
    sys.modules["antenv.axon_hooks"] = _stub
    antenv.axon_hooks = _stub

F32 = mybir.dt.float32
F32R = mybir.dt.float32r
BF16 = mybir.dt.bfloat16
EXP = mybir.ActivationFunctionType.Exp
MULT = mybir.AluOpType.mult
ADD = mybir.AluOpType.add

B, T, C, H = 4, 2048, 1024, 16
HD = C // H              # 64 head dim
N_CORES = 8
HPC = H // 2             # 8 heads per core
MPC = C // 2             # 512 channels per core
MT = MPC // 128          # 4 m-tiles per core
CT = C // 128            # 8 contraction tiles
TT = T // 128            # 16 key/time tiles of 128
QC = T // 512            # 4 query chunks of 512
SCALE = float(1.0 / np.sqrt(HD))
GRP = 3                  # key blocks fused per scores tile / exp instruction

_CACHE = {}


def _build(bench_loops=None, phases=(1, 2, 3)):
    import contextlib

    def _rep(tc):
        return tc.For_i(0, bench_loops, 1) if bench_loops else contextlib.nullcontext()

    nc = bacc.Bacc()
    xT = nc.declare_dram_parameter("xT", [C, T], BF16, isOutput=False)
    wqT = nc.declare_dram_parameter("wqT", [C, MPC], BF16, isOutput=False)
    wkT = nc.declare_dram_parameter("wkT", [C, MPC], BF16, isOutput=False)
    wvT = nc.declare_dram_parameter("wvT", [C, MPC], BF16, isOutput=False)
    wpT = nc.declare_dram_parameter("wpT", [MPC, C], BF16, isOutput=False)
    bqc = nc.declare_dram_parameter("bqc", [MPC, 1], F32, isOutput=False)
    bkc = nc.declare_dram_parameter("bkc", [MPC, 1], F32, isOutput=False)
    bvb = nc.declare_dram_parameter("bvb", [128, MPC], F32, isOutput=False)
    bpb = nc.declare_dram_parameter("bpb", [128, C], F32, isOutput=False)
    mask01d = nc.declare_dram_parameter("mask01", [128, 128], BF16, isOutput=False)
    outp = nc.declare_dram_parameter("out", [T, C], F32, isOutput=True)

    with tile.TileContext(nc) as tc:
        with ExitStack() as ctx:
            persist = ctx.enter_context(tc.tile_pool(name="persist", bufs=1))

            # ---- constants / biases ----
            mask01_sb = persist.tile([128, 128], BF16, name="mask01_sb")
            nc.sync.dma_start(out=mask01_sb, in_=mask01d[:, :])
            bq_col = persist.tile([128, MT], F32, name="bq_col")
            bk_col = persist.tile([128, MT], F32, name="bk_col")
            for m in range(MT):
                nc.sync.dma_start(out=bq_col[:, m:m + 1], in_=bqc[m * 128:(m + 1) * 128, :])
                nc.sync.dma_start(out=bk_col[:, m:m + 1], in_=bkc[m * 128:(m + 1) * 128, :])
            bv_bc = persist.tile([128, MPC], F32, name="bv_bc")
            nc.sync.dma_start(out=bv_bc, in_=bvb[:, :])
            bp_bc = persist.tile([128, C], F32, name="bp_bc")
            nc.sync.dma_start(out=bp_bc, in_=bpb[:, :])

            # ---- persistent activations (bf16) ----
            qT_sb = [persist.tile([128, T], BF16, name=f"qT{m}") for m in range(MT)]
            kT_sb = [persist.tile([128, T], BF16, name=f"kT{m}") for m in range(MT)]
            vAug = [persist.tile([128, HPC * (HD + 1)], BF16, name=f"vAug{t}") for t in range(TT)]
            yT_sb = [persist.tile([128, T], BF16, name=f"yT{m}") for m in range(MT)]

            # ones column of vAug never changes: set once
            ones8_f32 = persist.tile([128, HPC], F32, name="ones8_f32")
            nc.vector.memset(ones8_f32, 1.0)
            for t_ in range(TT):
                va = vAug[t_].rearrange("p (h w) -> p h w", w=HD + 1)
                nc.vector.tensor_copy(va[:, :, HD], ones8_f32)

            if 1 not in phases:
                # bench-only: fill phase-1 products with benign finite data
                half_f = persist.tile([128, HPC], F32, name="half_f")
                nc.vector.memset(half_f, 0.125)
                for m in range(MT):
                    nc.vector.memset(qT_sb[m], 0.125)
                    nc.vector.memset(kT_sb[m], 0.125)
                for t_ in range(TT):
                    va = vAug[t_].rearrange("p (h w) -> p h w", w=HD + 1)
                    for d in range(HD):
                        nc.vector.tensor_copy(va[:, :, d], half_f)
            if 3 in phases and 2 not in phases:
                for m in range(MT):
                    nc.vector.memset(yT_sb[m], 0.125)

            # ================= phase 1: projections =================
            with ExitStack() as ctx1:
              if 1 in phases:
                  pool_w = ctx1.enter_context(tc.tile_pool(name="pool_w", bufs=1))
                  pool_xs = ctx1.enter_context(tc.tile_pool(name="pool_xs", bufs=1))
                  pool_c1 = ctx1.enter_context(tc.tile_pool(name="pool_c1", bufs=2))
                  ps_1 = ctx1.enter_context(tc.tile_pool(name="ps_1", bufs=4, space="PSUM"))
                  wq_t = [pool_w.tile([128, MPC], BF16, name=f"wq{c}") for c in range(CT)]
                  wk_t = [pool_w.tile([128, MPC], BF16, name=f"wk{c}") for c in range(CT)]
                  wv_t = [pool_w.tile([128, MPC], BF16, name=f"wv{c}") for c in range(CT)]
                  for c in range(CT):
                      nc.sync.dma_start(out=wq_t[c], in_=wqT[c * 128:(c + 1) * 128, :])
                      nc.sync.dma_start(out=wk_t[c], in_=wkT[c * 128:(c + 1) * 128, :])
                      nc.sync.dma_start(out=wv_t[c], in_=wvT[c * 128:(c + 1) * 128, :])

                  rep1 = ctx1.enter_context(_rep(tc))
                  for tcb in range(2):          # 1024-col super-chunks
                      tb = tcb * 1024
                      xs = []
                      for c in range(CT):
                          x_ = pool_xs.tile([128, 1024], BF16, name=f"xs_{tcb}_{c}",
                                            tag="xs", bufs=12)
                          nc.sync.dma_start(out=x_, in_=xT[c * 128:(c + 1) * 128, tb:tb + 1024])
                          xs.append(x_)
                      for sub in range(2):      # 512-col compute steps
                          t0 = tb + sub * 512
                          s0 = sub * 512
                          # qT / kT (weight-stationary): psum[m 128, t 512]
                          for lbl, wt, bcol, dst in (("q", wq_t, bq_col, qT_sb),
                                                     ("k", wk_t, bk_col, kT_sb)):
                              for m in range(MT):
                                  ps = ps_1.tile([128, 512], F32, name=f"ps_{tcb}_{sub}_{lbl}_{m}",
                                                 tag="p1")
                                  for c in range(CT):
                                      nc.tensor.matmul(ps, wt[c][:, m * 128:(m + 1) * 128],
                                                       xs[c][:, s0:s0 + 512],
                                                       start=(c == 0), stop=(c == CT - 1))
                                  nc.vector.tensor_tensor(
                                      out=dst[m][:, t0:t0 + 512], in0=ps,
                                      in1=bcol[:, m:m + 1].to_broadcast([128, 512]), op=ADD)
                          # v (x-stationary): psum[t 128, m 512] -> vAug
                          for tt in range(4):
                              tg = (t0 // 128) + tt
                              ps = ps_1.tile([128, MPC], F32, name=f"psv_{tg}", tag="p1")
                              for c in range(CT):
                                  nc.tensor.matmul(ps, xs[c][:, s0 + tt * 128:s0 + (tt + 1) * 128],
                                                   wv_t[c], start=(c == 0), stop=(c == CT - 1))
                              va = vAug[tg].rearrange("p (h w) -> p h w", w=HD + 1)
                              nc.vector.tensor_tensor(
                                  out=va[:, :, 0:HD],
                                  in0=ps.rearrange("p (h w) -> p h w", w=HD),
                                  in1=bv_bc.rearrange("p (h w) -> p h w", w=HD), op=ADD)

            # ================= phase 2: attention =================
            with ExitStack() as ctx2:
              if 2 in phases:
                  pool_p = ctx2.enter_context(tc.tile_pool(name="pool_p", bufs=1))
                  pool_t2 = ctx2.enter_context(tc.tile_pool(name="pool_t2", bufs=2))
                  ps_sc = ctx2.enter_context(tc.tile_pool(name="ps_sc", bufs=2, space="PSUM"))
                  ps_y = ctx2.enter_context(tc.tile_pool(name="ps_y", bufs=2, space="PSUM"))
                  rep2 = ctx2.enter_context(_rep(tc))
                  for qc in range(QC):
                      q0 = qc * 512
                      jmax = 4 * qc + 3
                      # groups of up to GRP key blocks
                      groups = []
                      j = 0
                      while j <= jmax:
                          groups.append(list(range(j, min(j + GRP, jmax + 1))))
                          j += GRP
                      for h in range(HPC):
                          mt, so = h // 2, (h % 2) * 64
                          kT_h = kT_sb[mt]
                          qT_h = qT_sb[mt]
                          psy = ps_y.tile([HD + 1, 512], F32, name=f"psy_{h}_{qc}", tag="psy")

                          def _emit_y(grp, P):
                              for gi, j in enumerate(grp):
                                  lo = max(0, j * 128 - q0)
                                  nc.tensor.matmul(
                                      psy[:, lo:512],
                                      vAug[j].rearrange("p (h w) -> p h w", w=HD + 1)[:, h, :],
                                      P[:, gi * 512 + lo:(gi + 1) * 512],
                                      start=(j == 0), stop=(j == jmax))

                          pend = []
                          for grp in groups:
                              glo = max(0, grp[0] * 128 - q0)
                              gw = len(grp) * 512
                              sc = ps_sc.tile([128, GRP * 512], F32,
                                              name=f"sc_{h}_{qc}_{grp[0]}", tag="sc")
                              for gi, j in enumerate(grp):
                                  # non-first blocks write their full 512 cols
                                  # so the fused exp never reads stale PSUM
                                  # (the sub-lo cols are invalid but masked by
                                  # the y-matmul stream ranges)
                                  wlo = max(0, j * 128 - q0) if gi == 0 else 0
                                  nc.tensor.matmul(
                                      sc[:, gi * 512 + wlo:(gi + 1) * 512],
                                      kT_h[so:so + HD, j * 128:(j + 1) * 128],
                                      qT_h[so:so + HD, q0 + wlo:q0 + 512],
                                      start=True, stop=True)
                              P = pool_p.tile([128, GRP * 512], BF16,
                                              name=f"P_{h}_{qc}_{grp[0]}", tag="P", bufs=4)
                              nc.scalar.activation(out=P[:, glo:gw], in_=sc[:, glo:gw],
                                                   func=EXP, scale=SCALE)
                              for gi, j in enumerate(grp):
                                  if j >= 4 * qc:  # diagonal block: in-tile causal mask
                                      lo = j * 128 - q0
                                      nc.gpsimd.tensor_tensor(
                                          out=P[:, gi * 512 + lo:gi * 512 + lo + 128],
                                          in0=P[:, gi * 512 + lo:gi * 512 + lo + 128],
                                          in1=mask01_sb, op=MULT)
                              pend.append((grp, P))
                              if len(pend) > 2:
                                  _emit_y(*pend.pop(0))
                          for pd in pend:
                              _emit_y(*pd)
                          # normalize: recip of denominator row, partition
                          # broadcast, multiply into SBUF yT (bf16)
                          r32 = pool_t2.tile([1, 512], F32, name=f"r32_{h}_{qc}", tag="r32", bufs=3)
                          nc.vector.reciprocal(r32, psy[HD:HD + 1, :])
                          rb = pool_t2.tile([HD, 512], F32, name=f"rb_{h}_{qc}", tag="rb", bufs=3)
                          nc.gpsimd.partition_broadcast(rb, r32)
                          nc.vector.tensor_tensor(
                              out=yT_sb[mt][so:so + HD, q0:q0 + 512],
                              in0=psy[0:HD, :], in1=rb, op=MULT)

            # ================= phase 3: output projection =================
            with ExitStack() as ctx3:
              if 3 in phases:
                  pool_3 = ctx3.enter_context(tc.tile_pool(name="pool_3", bufs=1))
                  ps_o = ctx3.enter_context(tc.tile_pool(name="ps_o", bufs=2, space="PSUM"))
                  wp_t = [pool_3.tile([128, C], BF16, name=f"wp{m}") for m in range(MT)]
                  for m in range(MT):
                      nc.sync.dma_start(out=wp_t[m], in_=wpT[m * 128:(m + 1) * 128, :])
                  rep3 = ctx3.enter_context(_rep(tc))
                  for tt in range(TT):
                      o_sb = pool_3.tile([128, C], F32, name=f"o_{tt}", tag="o", bufs=3)
                      for nch in range(2):
                          n0 = nch * 512
                          ps = ps_o.tile([128, 512], F32, name=f"pso_{tt}_{nch}", tag="pso")
                          for m in range(MT):
                              nc.tensor.matmul(ps, yT_sb[m][:, tt * 128:(tt + 1) * 128],
                                               wp_t[m][:, n0:n0 + 512],
                                               start=(m == 0), stop=(m == MT - 1))
                          nc.vector.tensor_tensor(out=o_sb[:, n0:n0 + 512], in0=ps,
                                                  in1=bp_bc[:, n0:n0 + 512], op=ADD)
                      nc.sync.dma_start(out=outp[tt * 128:(tt + 1) * 128, :], in_=o_sb)
    nc.finalize()
    return nc


def _get_nc(bench_loops=None, phases=(1, 2, 3)):
    key = ("nc", bench_loops, tuple(phases))
    if key not in _CACHE:
        _CACHE[key] = _build(bench_loops, phases)
    return _CACHE[key]


def make_in_maps(x, Wk, bk, Wq, bq, Wv, bv, Wp, bp):
    import ml_dtypes

    x = np.asarray(x, dtype=np.float32)
    Wk, Wq, Wv, Wp = (np.asarray(a, dtype=np.float32) for a in (Wk, Wq, Wv, Wp))
    bk, bq, bv, bp = (np.asarray(a, dtype=np.float32) for a in (bk, bq, bv, bp))
    bf16 = ml_dtypes.bfloat16

    mask01 = np.where(np.tril(np.ones((128, 128), dtype=bool)).T, 1.0, 0.0).astype(bf16)
    xT_b = [np.ascontiguousarray(x[b].T).astype(bf16) for b in range(B)]
    in_maps = []
    for c in range(N_CORES):
        b, half = c // 2, c % 2
        hs = half * MPC
        in_maps.append({
            "xT": xT_b[b],
            "wqT": np.ascontiguousarray(Wq[hs:hs + MPC, :].T).astype(bf16),
            "wkT": np.ascontiguousarray(Wk[hs:hs + MPC, :].T).astype(bf16),
            "wvT": np.ascontiguousarray(Wv[hs:hs + MPC, :].T).astype(bf16),
            "wpT": np.ascontiguousarray(Wp[:, hs:hs + MPC].T).astype(bf16),
            "bqc": bq[hs:hs + MPC].reshape(MPC, 1).copy(),
            "bkc": bk[hs:hs + MPC].reshape(MPC, 1).copy(),
            "bvb": np.tile(bv[hs:hs + MPC].reshape(1, MPC), (128, 1)).copy(),
            "bpb": np.tile((bp if half == 0 else np.zeros_like(bp)).reshape(1, C),
                           (128, 1)).copy(),
            "mask01": mask01,
        })
    return in_maps


def kernel(x, Wk, bk, Wq, bq, Wv, bv, Wp, bp, **run_kwargs):
    in_maps = make_in_maps(x, Wk, bk, Wq, bq, Wv, bv, Wp, bp)
    nc = _get_nc()
    res = run_bass_kernel_spmd(nc, in_maps, core_ids=list(range(N_CORES)), **run_kwargs)
    out = np.empty((B, T, C), dtype=np.float32)
    for b in range(B):
        out[b] = res.results[2 * b]["out"] + res.results[2 * b + 1]["out"]
    if run_kwargs:
        kernel.last_results = res
    return out
